# revision 1
# baseline (speedup 1.0000x reference)
# Multi-head self-attention (B=2, T=2048, C=2048, H=16) on 8 trn2 NeuronCores.
# Sharding: core = (batch b, head-group g) with 4 heads per core.
# Inputs are pre-cast to bf16 and packed DMA-friendly on the host (the device
# would do the identical round-to-nearest cast before its bf16 matmuls).
# Per-core program (Tile framework, bf16 matmuls with fp32 PSUM accumulation):
#   qk^T = W_qk^T @ x^T   (lhsT = W chunks, rhs = x^T)      -> [D, T] per head
#   v    = x @ W_v        (lhsT = x^T chunks, rhs = W_v)    -> [T, D] natural
#   RoPE on q^T/k^T via half-swap DMA + elementwise mul/add (in place)
#   S^T tile = k_rope^T.T @ q_rope^T ; E^T = exp(scale*S^T) (causal)
#   out^T = v.T @ E^T ; rowsums via ones-matmul on DVE-paired E tiles
#   normalize off the PSUM path: evac unscaled, scale by bcast(1/sums) in SBUF
#   y_partial = out_heads^T.T @ W_p rows  -> [T, C], host sums 4 partials.
import sys

import numpy as np
import ml_dtypes

for _p in ("/opt/trn_rl_repo",):
    if _p not in sys.path:
        sys.path.append(_p)

import concourse.bass as bass
import concourse.mybir as mybir
import concourse.tile as tile
from concourse import bacc
from concourse.bass_utils import run_bass_kernel_spmd

P = 128
T = 2048
C = 2048
D = 128
NH = 4            # heads per core
KO = C // P       # 16 contraction chunks
TQ = 512          # q-tile width
NQ = T // TQ      # 4
NT = T // P       # 16 t-subtiles
SCALE = float(np.float32(1.0) / np.sqrt(np.float32(D)))

F32 = mybir.dt.float32
BF16 = mybir.dt.bfloat16
AF = mybir.ActivationFunctionType
BF = ml_dtypes.bfloat16

TRACE = False
_CACHED_NC = None


def _tri_mask_np():
    p = np.arange(P)[:, None]
    q = np.arange(P)[None, :]
    return (p <= q).astype(BF)


def build_nc():
    nc = bacc.Bacc("TRN2", target_bir_lowering=False, debug=False,
                   enable_asserts=False)

    # bf16 inputs, packed so every DMA moves >=4KB contiguous per partition
    xT_d = nc.dram_tensor("xT", [C, T], BF16, kind="ExternalInput")
    wqk_d = nc.dram_tensor("wqk", [8, P, KO, P], BF16, kind="ExternalInput")
    wv_d = nc.dram_tensor("wv", [P, KO, NH * D], BF16, kind="ExternalInput")
    wp_d = nc.dram_tensor("wp", [P, NH, C], BF16, kind="ExternalInput")
    cos_d = nc.dram_tensor("cosT", [D, T], BF16, kind="ExternalInput")
    sin_d = nc.dram_tensor("sinT", [D, T], BF16, kind="ExternalInput")
    y_d = nc.dram_tensor("y", [T, C], F32, kind="ExternalOutput")

    mask_d = nc.inline_tensor(_tri_mask_np(), name="trimask")

    xT = xT_d.ap().rearrange("(ko p) t -> p ko t", p=P)          # [128,16,2048]
    wqk = wqk_d.ap()
    y = y_d.ap()

    with tile.TileContext(nc) as tc:
        with (
            tc.tile_pool(name="glob", bufs=1) as glob,
            tc.tile_pool(name="rawp", bufs=1) as rawp,
        ):
            ones_sb = glob.tile([P, P], BF16, tag="ones")
            nc.vector.memset(ones_sb[:], 1.0)
            v_b = [glob.tile([P, NT, P], BF16, tag=f"v_b{h}", name=f"v_b{h}")
                   for h in range(NH)]
            raw = [rawp.tile([P, T], BF16, tag=f"raw{m}", name=f"raw{m}")
                   for m in range(8)]

            # =============== Phase B: qkv matmuls + RoPE ===============
            with tc.tile_pool(name="loadB", bufs=1) as lB, \
                 tc.tile_pool(name="shufp", bufs=1) as shufp, \
                 tc.tile_pool(name="psB", bufs=6, space="PSUM") as psB:

                xb = lB.tile([P, KO, T], BF16, tag="xb")
                wqk_b = [lB.tile([P, KO, P], BF16, tag=f"wqk_b{m}",
                                 name=f"wqk_b{m}") for m in range(8)]
                # interleave weight/x loads; arrival order matches use order
                nc.sync.dma_start(wqk_b[0][:, 0:4, :], wqk[0, :, 0:4, :])
                nc.sync.dma_start(xb[:, 0, :], xT[:, 0, :])
                nc.sync.dma_start(wqk_b[0][:, 4:KO, :], wqk[0, :, 4:KO, :])
                for m in range(1, 8):
                    nc.sync.dma_start(wqk_b[m][:], wqk[m])
                    nc.sync.dma_start(xb[:, m, :], xT[:, m, :])
                for ko in range(8, KO):
                    nc.sync.dma_start(xb[:, ko, :], xT[:, ko, :])
                cos_b = lB.tile([P, T], BF16, tag="cos_b")
                sin_b = lB.tile([P, T], BF16, tag="sin_b")
                nc.sync.dma_start(cos_b[:], cos_d.ap())
                nc.sync.dma_start(sin_b[:], sin_d.ap())
                wv_b = lB.tile([P, KO, NH * D], BF16, tag="wv_b")
                nc.sync.dma_start(wv_b[:], wv_d.ap())

                # qk matmuls; k-order staggered by m so each group consumes
                # chunks roughly in DMA-arrival order
                for m in range(8):
                    pss = [psB.tile([P, TQ], F32, tag="psBig", name="psqk")
                           for _ in range(NQ)]
                    kos = [(2 * m + i) % KO for i in range(KO)]
                    for i, ko in enumerate(kos):
                        for n in range(NQ):
                            nc.tensor.matmul(
                                pss[n][:], lhsT=wqk_b[m][:, ko, :],
                                rhs=xb[:, ko, n * TQ:(n + 1) * TQ],
                                start=(i == 0), stop=(i == KO - 1))
                    for n in range(NQ):
                        nc.scalar.activation(
                            raw[m][:, n * TQ:(n + 1) * TQ], pss[n][:], AF.Copy)
                    # rope in place
                    r = raw[m]
                    sh = shufp.tile([P, T], BF16, tag="shuf", name="sh")
                    nc.sync.dma_start(sh[0:64, :], r[64:128, :])
                    nc.sync.dma_start(sh[64:128, :], r[0:64, :])
                    nc.vector.tensor_mul(sh[:], sh[:], sin_b[:])
                    nc.vector.tensor_mul(r[:], r[:], cos_b[:])
                    nc.vector.tensor_add(r[:], r[:], sh[:])

                # v matmuls (natural layout)
                for t in range(NT):
                    psv = psB.tile([P, TQ], F32, tag="psBig", name="psv")
                    for ko in range(KO):
                        nc.tensor.matmul(
                            psv[:], lhsT=xb[:, ko, t * P:(t + 1) * P],
                            rhs=wv_b[:, ko, :],
                            start=(ko == 0), stop=(ko == KO - 1))
                    for h in range(NH):
                        nc.scalar.activation(
                            v_b[h][:, t, :], psv[:, h * P:(h + 1) * P], AF.Copy)

            # =============== Phase D: attention ===============
            with tc.tile_pool(name="attnp", bufs=1) as ap_, \
                 tc.tile_pool(name="etp", bufs=3) as etp, \
                 tc.tile_pool(name="nrm", bufs=2) as nrm:

                mask_sb = ap_.tile([P, P], BF16, tag="trimask")
                nc.sync.dma_start(mask_sb[:], mask_d.ap())
                outT = [ap_.tile([P, T], BF16, tag=f"outT{h}", name=f"outT{h}")
                        for h in range(NH)]
                wp_b = ap_.tile([P, NH, C], BF16, tag="wp_b")
                nc.sync.dma_start(wp_b[:], wp_d.ap())

                with tc.tile_pool(name="psS2", bufs=2, space="PSUM") as psS2, \
                     tc.tile_pool(name="psO", bufs=3, space="PSUM") as psO, \
                     tc.tile_pool(name="psR", bufs=1, space="PSUM") as psR:
                    for h in range(NH):
                        qr = raw[h]
                        kr = raw[4 + h]
                        # qo descending: dense large-qo tiles first, so the
                        # latency-bound qo=0 chain overlaps other work
                        for qo in reversed(range(NQ)):
                            qsl = slice(qo * TQ, (qo + 1) * TQ)
                            nfull = 4 * qo
                            ps_o = psO.tile([P, TQ], F32, tag="psout",
                                            name="ps_o")
                            ps_r = psR.tile([P, TQ], F32, tag="psrow",
                                            name="ps_r")
                            # diagonal chunks first: their exp/mask latency
                            # chains hide under the dense pairs that follow
                            for jr in range(4):
                                j = nfull + jr
                                off = jr * P
                                ps2 = psS2.tile([P, 2, TQ], F32, tag="psscore",
                                                name="ps2d")
                                nc.tensor.matmul(
                                    ps2[:, 0, off:TQ],
                                    lhsT=kr[:, j * P:(j + 1) * P],
                                    rhs=qr[:, qo * TQ + off:(qo + 1) * TQ],
                                    start=True, stop=True)
                                et = etp.tile([P, TQ], BF16, tag="et1",
                                              name="et1", bufs=6)
                                nc.scalar.activation(et[:, off:TQ],
                                                     ps2[:, 0, off:TQ], AF.Exp,
                                                     scale=SCALE)
                                nc.vector.tensor_mul(et[:, off:off + P],
                                                     et[:, off:off + P],
                                                     mask_sb[:])
                                last = (jr == 3) and nfull == 0
                                nc.tensor.matmul(
                                    ps_o[:, off:TQ], lhsT=v_b[h][:, j, :],
                                    rhs=et[:, off:TQ],
                                    start=(jr == 0), stop=last)
                                nc.tensor.matmul(
                                    ps_r[:, off:TQ], lhsT=ones_sb[:],
                                    rhs=et[:, off:TQ],
                                    start=(jr == 0), stop=last)
                            for pr in range(nfull // 2):
                                ps2 = psS2.tile([P, 2, TQ], F32, tag="psscore",
                                                name="ps2")
                                for s in range(2):
                                    j = 2 * pr + s
                                    nc.tensor.matmul(
                                        ps2[:, s, :],
                                        lhsT=kr[:, j * P:(j + 1) * P],
                                        rhs=qr[:, qsl], start=True, stop=True)
                                et2 = etp.tile([P, 2, TQ], BF16, tag="et2",
                                               name="et2", bufs=5)
                                nc.scalar.activation(et2[:], ps2[:], AF.Exp,
                                                     scale=SCALE)
                                last = (pr == nfull // 2 - 1)
                                for s in range(2):
                                    j = 2 * pr + s
                                    nc.tensor.matmul(
                                        ps_o[:], lhsT=v_b[h][:, j, :],
                                        rhs=et2[:, s, :],
                                        start=False, stop=(last and s == 1))
                                esum = etp.tile([P, TQ], BF16, tag="esum",
                                                name="esum", bufs=3)
                                nc.vector.tensor_add(esum[:], et2[:, 0, :],
                                                     et2[:, 1, :])
                                nc.tensor.matmul(
                                    ps_r[:], lhsT=ones_sb[:],
                                    rhs=esum[:],
                                    start=False, stop=last)
                            # rowsums arrive replicated on all partitions
                            # (ones lhsT is [128,128]) - no broadcast needed
                            sums = nrm.tile([P, TQ], F32, tag="sums",
                                            name="sums")
                            nc.vector.tensor_copy(sums[:], ps_r[:])
                            recip = nrm.tile([P, TQ], F32, tag="recip",
                                             name="recip")
                            nc.vector.reciprocal_approx_fast(recip[:], sums[:])
                            nc.vector.tensor_mul(outT[h][:, qsl], ps_o[:],
                                                 recip[:])

                # =============== Phase F: projection ===============
                with tc.tile_pool(name="psPj", bufs=6, space="PSUM") as psPj, \
                     tc.tile_pool(name="ystg", bufs=3) as ystg:
                    for t in range(NT):
                        pss = [psPj.tile([P, TQ], F32, tag="psproj",
                                         name="psy") for _ in range(NQ)]
                        for h in range(NH):
                            for cn in range(NQ):
                                nc.tensor.matmul(
                                    pss[cn][:],
                                    lhsT=outT[h][:, t * P:(t + 1) * P],
                                    rhs=wp_b[:, h, cn * TQ:(cn + 1) * TQ],
                                    start=(h == 0), stop=(h == NH - 1))
                        ys = ystg.tile([P, T], F32, tag="ystage", name="ys")
                        for cn in range(NQ):
                            nc.scalar.activation(
                                ys[:, cn * TQ:(cn + 1) * TQ], pss[cn][:],
                                AF.Copy)
                        eng = nc.sync if t % 2 == 0 else nc.scalar
                        eng.dma_start(y[t * P:(t + 1) * P, :], ys[:])

    nc.compile()
    return nc


def _get_nc():
    global _CACHED_NC
    if _CACHED_NC is None:
        _CACHED_NC = build_nc()
    return _CACHED_NC


LAST_RESULTS = None


def kernel(x, cos, sin, W_attn, W_proj):
    global LAST_RESULTS
    x = np.asarray(x, np.float32)
    cos = np.asarray(cos, np.float32)
    sin = np.asarray(sin, np.float32)
    W_attn = np.asarray(W_attn, np.float32)
    W_proj = np.asarray(W_proj, np.float32)
    B = x.shape[0]

    cosT = np.ascontiguousarray(cos.T).astype(BF)          # [D, T]
    sinTf = np.ascontiguousarray(sin.T).copy()
    sinTf[: D // 2] *= -1.0                                # sign-folded rotate
    sinT = sinTf.astype(BF)

    xTs = [np.ascontiguousarray(x[b].T).astype(BF) for b in range(B)]
    in_maps = []
    for b in range(B):
        for g in range(4):
            csl = slice(g * 512, (g + 1) * 512)
            wqk2 = np.concatenate([W_attn[:, csl], W_attn[:, C:][:, csl]],
                                  axis=1).astype(BF)       # [C, 1024]
            # pack [8, 128, 16, 128]: wqkr[m, p, ko, j] = wqk2[128*ko+p, 128*m+j]
            wqkr = np.ascontiguousarray(
                wqk2.reshape(KO, P, 8, P).transpose(2, 1, 0, 3))
            wv2 = W_attn[:, 2 * C:][:, csl].astype(BF)     # [C, 512]
            wvr = np.ascontiguousarray(
                wv2.reshape(KO, P, NH * D).transpose(1, 0, 2))  # [128,16,512]
            wp2 = W_proj[g * 512:(g + 1) * 512, :].astype(BF)   # [512, C]
            wpr = np.ascontiguousarray(
                wp2.reshape(NH, P, C).transpose(1, 0, 2))       # [128,4,2048]
            in_maps.append({"xT": xTs[b], "wqk": wqkr, "wv": wvr, "wp": wpr,
                            "cosT": cosT, "sinT": sinT})

    nc = _get_nc()
    res = run_bass_kernel_spmd(nc, in_maps, core_ids=list(range(8)),
                               trace=TRACE)
    LAST_RESULTS = res

    out = np.zeros((B, T, C), np.float32)
    for b in range(B):
        acc = res.results[b * 4 + 0]["y"].astype(np.float32)
        for g in range(1, 4):
            acc = acc + res.results[b * 4 + g]["y"]
        out[b] = acc
    return out



# revision 2
# speedup vs baseline: 1.0303x; 1.0303x over previous
# Multi-head self-attention (B=2, T=2048, C=2048, H=16) on 8 trn2 NeuronCores.
# Sharding: core = (batch b, head-group g) with 4 heads per core.
# Inputs are pre-cast to bf16 and packed DMA-friendly on the host (the device
# would do the identical round-to-nearest cast before its bf16 matmuls).
# Per-core program (Tile framework, bf16 matmuls with fp32 PSUM accumulation):
#   all input DMAs issued from the (otherwise idle) GPSIMD queue, in
#   consumption order; first two qk m-groups interleaved so early tensor
#   demand matches DMA supply rate
#   qk^T = W_qk^T @ x^T   (lhsT = W chunks, rhs = x^T)      -> [D, T] per head
#   v    = x @ W_v        (lhsT = x^T chunks, rhs = W_v)    -> [T, D] natural
#   RoPE on q^T/k^T via half-swap DMA + elementwise mul/add (in place)
#   S^T tile = k_rope^T.T @ q_rope^T ; E^T = exp(scale*S^T) (causal)
#   diag S-matmuls batched ahead of the O/R matmuls (hides exp latency)
#   out^T = v.T @ E^T ; rowsums via ones-matmul on DVE-paired E tiles
#   normalize off the PSUM path: recip straight from PSUM, scale in SBUF
#   y_partial = out_heads^T.T @ W_p rows  -> [T, C], host sums 4 partials.
import sys

import numpy as np
import ml_dtypes

for _p in ("/opt/trn_rl_repo",):
    if _p not in sys.path:
        sys.path.append(_p)

import concourse.bass as bass
import concourse.mybir as mybir
import concourse.tile as tile
from concourse import bacc
from concourse.bass_utils import run_bass_kernel_spmd

P = 128
T = 2048
C = 2048
D = 128
NH = 4            # heads per core
KO = C // P       # 16 contraction chunks
TQ = 512          # q-tile width
NQ = T // TQ      # 4
NT = T // P       # 16 t-subtiles
SCALE = float(np.float32(1.0) / np.sqrt(np.float32(D)))

F32 = mybir.dt.float32
BF16 = mybir.dt.bfloat16
AF = mybir.ActivationFunctionType
BF = ml_dtypes.bfloat16

TRACE = False
_CACHED_NC = None


def _tri_mask_np():
    p = np.arange(P)[:, None]
    q = np.arange(P)[None, :]
    return (p <= q).astype(BF)


def build_nc():
    nc = bacc.Bacc("TRN2", target_bir_lowering=False, debug=False,
                   enable_asserts=False)

    # bf16 inputs, packed so every DMA moves >=4KB contiguous per partition
    xT_d = nc.dram_tensor("xT", [C, T], BF16, kind="ExternalInput")
    wqk_d = nc.dram_tensor("wqk", [8, P, KO, P], BF16, kind="ExternalInput")
    wv_d = nc.dram_tensor("wv", [P, KO, NH * D], BF16, kind="ExternalInput")
    wp_d = nc.dram_tensor("wp", [P, NH, C], BF16, kind="ExternalInput")
    cos_d = nc.dram_tensor("cosT", [D, T], BF16, kind="ExternalInput")
    sin_d = nc.dram_tensor("sinT", [D, T], BF16, kind="ExternalInput")
    y_d = nc.dram_tensor("y", [T, C], F32, kind="ExternalOutput")

    mask_d = nc.inline_tensor(_tri_mask_np(), name="trimask")

    xT = xT_d.ap().rearrange("(ko p) t -> p ko t", p=P)          # [128,16,2048]
    wqk = wqk_d.ap()
    y = y_d.ap()

    with tile.TileContext(nc) as tc:
        with (
            tc.tile_pool(name="glob", bufs=1) as glob,
            tc.tile_pool(name="rawp", bufs=1) as rawp,
        ):
            ones_sb = glob.tile([P, P], BF16, tag="ones")
            nc.vector.memset(ones_sb[:], 1.0)
            # v for all 4 heads in one tile: v_b[:, t, h*P:(h+1)*P]
            v_b = glob.tile([P, NT, NH * P], BF16, tag="v_b")
            raw = [rawp.tile([P, T], BF16, tag=f"raw{m}", name=f"raw{m}")
                   for m in range(8)]
            mask_sb = glob.tile([P, P], BF16, tag="trimask")
            wp_b = glob.tile([P, NH, C], BF16, tag="wp_b")

            # =============== Phase B: qkv matmuls + RoPE ===============
            with tc.tile_pool(name="loadB", bufs=1) as lB, \
                 tc.tile_pool(name="shufp", bufs=1) as shufp, \
                 tc.tile_pool(name="psB", bufs=8, space="PSUM") as psB:

                xb = lB.tile([P, KO, T], BF16, tag="xb")
                wqk_b = [lB.tile([P, KO, P], BF16, tag=f"wqk_b{m}",
                                 name=f"wqk_b{m}") for m in range(8)]
                cos_b = lB.tile([P, T], BF16, tag="cos_b")
                sin_b = lB.tile([P, T], BF16, tag="sin_b")
                wv_b = lB.tile([P, KO, NH * D], BF16, tag="wv_b")

                # all loads issued from the idle gpsimd queue (cheap issue),
                # ordered by first-use time
                dma = nc.gpsimd.dma_start
                dma(wqk_b[0][:, 0:2, :], wqk[0, :, 0:2, :])
                dma(wqk_b[1][:, 0:2, :], wqk[1, :, 0:2, :])
                dma(xb[:, 0, :], xT[:, 0, :])
                dma(wqk_b[0][:, 2:KO, :], wqk[0, :, 2:KO, :])
                dma(wqk_b[1][:, 2:KO, :], wqk[1, :, 2:KO, :])
                dma(xb[:, 1, :], xT[:, 1, :])
                dma(cos_b[:], cos_d.ap())
                dma(sin_b[:], sin_d.ap())
                for ko in range(2, KO):
                    dma(xb[:, ko, :], xT[:, ko, :])
                    # stage the remaining weight groups between x chunks,
                    # each well before its first use at ~13.6*(m-1) us
                    if ko in (5, 8, 11, 13):
                        m = {5: 2, 8: 3, 11: 4, 13: 5}[ko]
                        dma(wqk_b[m][:], wqk[m])
                dma(wqk_b[6][:], wqk[6])
                dma(wqk_b[7][:], wqk[7])
                dma(wv_b[:], wv_d.ap())
                dma(mask_sb[:], mask_d.ap())
                dma(wp_b[:], wp_d.ap())

                def qk_evac_rope(m, pss):
                    for n in range(NQ):
                        nc.scalar.activation(
                            raw[m][:, n * TQ:(n + 1) * TQ], pss[n][:], AF.Copy)
                    r = raw[m]
                    sh = shufp.tile([P, T], BF16, tag="shuf", name="sh")
                    nc.sync.dma_start(sh[0:64, :], r[64:128, :])
                    nc.sync.dma_start(sh[64:128, :], r[0:64, :])
                    nc.vector.tensor_mul(sh[:], sh[:], sin_b[:])
                    nc.vector.tensor_mul(r[:], r[:], cos_b[:])
                    nc.vector.tensor_add(r[:], r[:], sh[:])

                # m=0,1 interleaved: each x chunk feeds 8 matmuls, matching
                # the DMA supply rate during the cold start
                pss01 = [[psB.tile([P, TQ], F32, tag="psBig", name="psqk")
                          for _ in range(NQ)] for _ in range(2)]
                for ko in range(KO):
                    for g in range(2):
                        for n in range(NQ):
                            nc.tensor.matmul(
                                pss01[g][n][:], lhsT=wqk_b[g][:, ko, :],
                                rhs=xb[:, ko, n * TQ:(n + 1) * TQ],
                                start=(ko == 0), stop=(ko == KO - 1))
                qk_evac_rope(0, pss01[0])
                qk_evac_rope(1, pss01[1])

                for m in range(2, 8):
                    pss = [psB.tile([P, TQ], F32, tag="psBig", name="psqk")
                           for _ in range(NQ)]
                    for ko in range(KO):
                        for n in range(NQ):
                            nc.tensor.matmul(
                                pss[n][:], lhsT=wqk_b[m][:, ko, :],
                                rhs=xb[:, ko, n * TQ:(n + 1) * TQ],
                                start=(ko == 0), stop=(ko == KO - 1))
                    qk_evac_rope(m, pss)

                # v matmuls (natural layout), single merged evac per t
                for t in range(NT):
                    psv = psB.tile([P, TQ], F32, tag="psBig", name="psv")
                    for ko in range(KO):
                        nc.tensor.matmul(
                            psv[:], lhsT=xb[:, ko, t * P:(t + 1) * P],
                            rhs=wv_b[:, ko, :],
                            start=(ko == 0), stop=(ko == KO - 1))
                    nc.scalar.activation(v_b[:, t, :], psv[:], AF.Copy)

            # =============== Phase D: attention ===============
            with tc.tile_pool(name="etp", bufs=3) as etp, \
                 tc.tile_pool(name="nrm", bufs=2) as nrm:

                outT = [glob.tile([P, T], BF16, tag=f"outT{h}", name=f"outT{h}")
                        for h in range(NH)]

                with tc.tile_pool(name="psS2", bufs=2, space="PSUM") as psS2, \
                     tc.tile_pool(name="psO", bufs=3, space="PSUM") as psO, \
                     tc.tile_pool(name="psR", bufs=1, space="PSUM") as psR:
                    for h in range(NH):
                        qr = raw[h]
                        kr = raw[4 + h]
                        vh = v_b[:, :, h * P:(h + 1) * P]
                        # qo descending: dense large-qo tiles first, so the
                        # latency-bound qo=0 chain overlaps other work
                        for qo in reversed(range(NQ)):
                            qsl = slice(qo * TQ, (qo + 1) * TQ)
                            nfull = 4 * qo
                            ps_o = psO.tile([P, TQ], F32, tag="psout",
                                            name="ps_o")
                            ps_r = psR.tile([P, TQ], F32, tag="psrow",
                                            name="ps_r")
                            # diagonal chunks: all 4 S-matmuls first, then
                            # the exp-dependent O/R matmuls (S3..S1 hide the
                            # exp latency of jr=0)
                            ps2a = psS2.tile([P, 2, TQ], F32, tag="psscore",
                                             name="ps2da")
                            ps2b = psS2.tile([P, 2, TQ], F32, tag="psscore",
                                             name="ps2db")
                            ets = []
                            for jr in range(4):
                                j = nfull + jr
                                off = jr * P
                                psd = (ps2a, ps2b)[jr // 2]
                                nc.tensor.matmul(
                                    psd[:, jr % 2, off:TQ],
                                    lhsT=kr[:, j * P:(j + 1) * P],
                                    rhs=qr[:, qo * TQ + off:(qo + 1) * TQ],
                                    start=True, stop=True)
                                et = etp.tile([P, TQ], BF16, tag="et1",
                                              name="et1", bufs=6)
                                nc.scalar.activation(et[:, off:TQ],
                                                     psd[:, jr % 2, off:TQ],
                                                     AF.Exp, scale=SCALE)
                                nc.vector.tensor_mul(et[:, off:off + P],
                                                     et[:, off:off + P],
                                                     mask_sb[:])
                                ets.append(et)
                            for jr in range(4):
                                j = nfull + jr
                                off = jr * P
                                last = (jr == 3) and nfull == 0
                                nc.tensor.matmul(
                                    ps_o[:, off:TQ], lhsT=vh[:, j, :],
                                    rhs=ets[jr][:, off:TQ],
                                    start=(jr == 0), stop=last)
                                nc.tensor.matmul(
                                    ps_r[:, off:TQ], lhsT=ones_sb[:],
                                    rhs=ets[jr][:, off:TQ],
                                    start=(jr == 0), stop=last)
                            # dense pairs with one-pair lookahead: S-matmuls
                            # of pair p+1 are emitted before O/R of pair p
                            npair = nfull // 2
                            pend = None
                            for pr in range(npair + 1):
                                if pr < npair:
                                    ps2 = psS2.tile([P, 2, TQ], F32,
                                                    tag="psscore", name="ps2")
                                    for s in range(2):
                                        j = 2 * pr + s
                                        nc.tensor.matmul(
                                            ps2[:, s, :],
                                            lhsT=kr[:, j * P:(j + 1) * P],
                                            rhs=qr[:, qsl],
                                            start=True, stop=True)
                                    et2 = etp.tile([P, 2, TQ], BF16, tag="et2",
                                                   name="et2", bufs=5)
                                    nc.scalar.activation(et2[:], ps2[:],
                                                         AF.Exp, scale=SCALE)
                                else:
                                    et2 = None
                                if pend is not None:
                                    et2p, prp = pend
                                    last = (prp == npair - 1)
                                    for s in range(2):
                                        j = 2 * prp + s
                                        nc.tensor.matmul(
                                            ps_o[:], lhsT=vh[:, j, :],
                                            rhs=et2p[:, s, :],
                                            start=False, stop=(last and s == 1))
                                    esum = etp.tile([P, TQ], BF16, tag="esum",
                                                    name="esum", bufs=3)
                                    nc.vector.tensor_add(esum[:], et2p[:, 0, :],
                                                         et2p[:, 1, :])
                                    nc.tensor.matmul(
                                        ps_r[:], lhsT=ones_sb[:],
                                        rhs=esum[:],
                                        start=False, stop=last)
                                pend = (et2, pr) if et2 is not None else None
                            # rowsums arrive replicated on all partitions;
                            # reciprocal straight from PSUM frees ps_r early
                            recip = nrm.tile([P, TQ], F32, tag="recip",
                                             name="recip")
                            nc.vector.reciprocal_approx_fast(recip[:], ps_r[:])
                            nc.vector.tensor_mul(outT[h][:, qsl], ps_o[:],
                                                 recip[:])

                # =============== Phase F: projection ===============
                with tc.tile_pool(name="psPj", bufs=3, space="PSUM") as psPj, \
                     tc.tile_pool(name="ystg", bufs=2) as ystg:
                    for t in range(NT):
                        ys = ystg.tile([P, T], F32, tag="ystage", name="ys")
                        for half in range(2):
                            psp = psPj.tile([P, 2, TQ], F32, tag="psproj",
                                            name="psy")
                            for h in range(NH):
                                for s in range(2):
                                    cn = 2 * half + s
                                    nc.tensor.matmul(
                                        psp[:, s, :],
                                        lhsT=outT[h][:, t * P:(t + 1) * P],
                                        rhs=wp_b[:, h, cn * TQ:(cn + 1) * TQ],
                                        start=(h == 0), stop=(h == NH - 1))
                            hs = slice(half * 2 * TQ, (half + 1) * 2 * TQ)
                            if half == 0:
                                nc.scalar.activation(ys[:, hs], psp[:],
                                                     AF.Copy)
                            else:
                                nc.vector.tensor_copy(ys[:, hs], psp[:])
                            nc.gpsimd.dma_start(
                                y[t * P:(t + 1) * P, hs], ys[:, hs])

    nc.compile()
    return nc


def _get_nc():
    global _CACHED_NC
    if _CACHED_NC is None:
        _CACHED_NC = build_nc()
    return _CACHED_NC


LAST_RESULTS = None


def kernel(x, cos, sin, W_attn, W_proj):
    global LAST_RESULTS
    x = np.asarray(x, np.float32)
    cos = np.asarray(cos, np.float32)
    sin = np.asarray(sin, np.float32)
    W_attn = np.asarray(W_attn, np.float32)
    W_proj = np.asarray(W_proj, np.float32)
    B = x.shape[0]

    cosT = np.ascontiguousarray(cos.T).astype(BF)          # [D, T]
    sinTf = np.ascontiguousarray(sin.T).copy()
    sinTf[: D // 2] *= -1.0                                # sign-folded rotate
    sinT = sinTf.astype(BF)

    xTs = [np.ascontiguousarray(x[b].T).astype(BF) for b in range(B)]
    in_maps = []
    for b in range(B):
        for g in range(4):
            csl = slice(g * 512, (g + 1) * 512)
            wqk2 = np.concatenate([W_attn[:, csl], W_attn[:, C:][:, csl]],
                                  axis=1).astype(BF)       # [C, 1024]
            # pack [8, 128, 16, 128]: wqkr[m, p, ko, j] = wqk2[128*ko+p, 128*m+j]
            wqkr = np.ascontiguousarray(
                wqk2.reshape(KO, P, 8, P).transpose(2, 1, 0, 3))
            wv2 = W_attn[:, 2 * C:][:, csl].astype(BF)     # [C, 512]
            wvr = np.ascontiguousarray(
                wv2.reshape(KO, P, NH * D).transpose(1, 0, 2))  # [128,16,512]
            wp2 = W_proj[g * 512:(g + 1) * 512, :].astype(BF)   # [512, C]
            wpr = np.ascontiguousarray(
                wp2.reshape(NH, P, C).transpose(1, 0, 2))       # [128,4,2048]
            in_maps.append({"xT": xTs[b], "wqk": wqkr, "wv": wvr, "wp": wpr,
                            "cosT": cosT, "sinT": sinT})

    nc = _get_nc()
    res = run_bass_kernel_spmd(nc, in_maps, core_ids=list(range(8)),
                               trace=TRACE)
    LAST_RESULTS = res

    out = np.zeros((B, T, C), np.float32)
    for b in range(B):
        acc = res.results[b * 4 + 0]["y"].astype(np.float32)
        for g in range(1, 4):
            acc = acc + res.results[b * 4 + g]["y"]
        out[b] = acc
    return out


# revision 6
# speedup vs baseline: 1.0630x; 1.0318x over previous
# Multi-head self-attention (B=2, T=2048, C=2048, H=16) on 8 trn2 NeuronCores.
# Sharding: core = (batch b, head-group g) with 4 heads per core.
# Inputs are pre-cast to bf16 and packed DMA-friendly on the host (the device
# would do the identical round-to-nearest cast before its bf16 matmuls).
# Per-core program (Tile framework, bf16 matmuls with fp32 PSUM accumulation):
#   input DMAs: first pieces issued in parallel from sync/scalar/vector,
#   bulk from the gpsimd queue in consumption order; first two qk m-groups
#   interleaved so early tensor demand matches DMA supply rate
#   qk^T = W_qk^T @ x^T   (lhsT = W chunks, rhs = x^T)      -> [D, T] per head
#   v    = x @ W_v        (lhsT = x^T chunks, rhs = W_v)    -> [T, D] natural
#   RoPE on q^T/k^T via half-swap DMA + elementwise mul/add (in place)
#   attention runs qo-outer/head-inner; projection tiles of the previous
#   qo-group are woven between head sections to keep the PE busy while the
#   activation engine works through the exps
#   S^T tile = k_rope^T.T @ q_rope^T ; E^T = exp(scale*S^T) (causal)
#   diag masking via [P,4,TQ] templates that also zero invalid columns, so
#   all E tiles of an (h,qo) sum on the DVE into one accumulator and a
#   single ones-matmul produces the softmax denominators
#   normalize off the PSUM path: recip straight from PSUM, scale in SBUF
#   y_partial = out_heads^T.T @ W_p rows  -> [T, C], host sums 4 partials.
import sys

import numpy as np
import ml_dtypes

for _p in ("/opt/trn_rl_repo",):
    if _p not in sys.path:
        sys.path.append(_p)

import concourse.bass as bass
import concourse.mybir as mybir
import concourse.tile as tile
from concourse import bacc
from concourse.bass_utils import run_bass_kernel_spmd

P = 128
T = 2048
C = 2048
D = 128
NH = 4            # heads per core
KO = C // P       # 16 contraction chunks
TQ = 512          # q-tile width
NQ = T // TQ      # 4
NT = T // P       # 16 t-subtiles
SCALE = float(np.float32(1.0) / np.sqrt(np.float32(D)))

F32 = mybir.dt.float32
BF16 = mybir.dt.bfloat16
AF = mybir.ActivationFunctionType
BF = ml_dtypes.bfloat16

TRACE = False
_CACHED_NC = None


def _tmpl_np():
    # tmpl[p, jr, c]: 0 for c < jr*P (invalid, also zeroes stale-garbage
    # columns so full-tile E sums are exact), causal tri on the diagonal
    # square, 1 beyond it
    t = np.zeros((P, 4, TQ), np.float32)
    p = np.arange(P)[:, None]
    c = np.arange(P)[None, :]
    tri = (p <= c).astype(np.float32)
    for jr in range(4):
        off = jr * P
        t[:, jr, off:off + P] = tri
        t[:, jr, off + P:] = 1.0
    return t.astype(BF)


def build_nc():
    nc = bacc.Bacc("TRN2", target_bir_lowering=False, debug=False,
                   enable_asserts=False)

    # bf16 inputs, packed so every DMA moves >=4KB contiguous per partition
    xT_d = nc.dram_tensor("xT", [C, T], BF16, kind="ExternalInput")
    wqk_d = nc.dram_tensor("wqk", [8, P, KO, P], BF16, kind="ExternalInput")
    wv_d = nc.dram_tensor("wv", [P, KO, NH * D], BF16, kind="ExternalInput")
    wp_d = nc.dram_tensor("wp", [P, NH, C], BF16, kind="ExternalInput")
    cos_d = nc.dram_tensor("cosT", [D, T], BF16, kind="ExternalInput")
    sin_d = nc.dram_tensor("sinT", [D, T], BF16, kind="ExternalInput")
    y_d = nc.dram_tensor("y", [T, C], F32, kind="ExternalOutput")

    tmpl_d = nc.inline_tensor(_tmpl_np(), name="trimaskt")

    xT = xT_d.ap().rearrange("(ko p) t -> p ko t", p=P)          # [128,16,2048]
    wqk = wqk_d.ap()
    y = y_d.ap()

    with tile.TileContext(nc) as tc:
        with (
            tc.tile_pool(name="glob", bufs=1) as glob,
            tc.tile_pool(name="rawp", bufs=1) as rawp,
        ):
            ones_sb = glob.tile([P, P], BF16, tag="ones")
            nc.vector.memset(ones_sb[:], 1.0)
            # v for all 4 heads in one tile: v_b[:, t, h*P:(h+1)*P]
            v_b = glob.tile([P, NT, NH * P], BF16, tag="v_b")
            raw = [rawp.tile([P, T], BF16, tag=f"raw{m}", name=f"raw{m}")
                   for m in range(8)]
            tmpl_sb = glob.tile([P, 4, TQ], BF16, tag="trimaskt")
            wp_b = glob.tile([P, NH, C], BF16, tag="wp_b")

            # =============== Phase B: qkv matmuls + RoPE ===============
            with tc.tile_pool(name="loadB", bufs=1) as lB, \
                 tc.tile_pool(name="shufp", bufs=1) as shufp, \
                 tc.tile_pool(name="psB", bufs=8, space="PSUM") as psB:

                xb = lB.tile([P, KO, T], BF16, tag="xb")
                wqk_b = [lB.tile([P, KO, P], BF16, tag=f"wqk_b{m}",
                                 name=f"wqk_b{m}") for m in range(8)]
                cos_b = lB.tile([P, T], BF16, tag="cos_b")
                sin_b = lB.tile([P, T], BF16, tag="sin_b")
                wv_b = lB.tile([P, KO, NH * D], BF16, tag="wv_b")

                # first pieces in parallel across the HWDGE queues so the
                # first matmul's inputs arrive with one issue latency
                nc.sync.dma_start(wqk_b[0][:, 0:2, :], wqk[0, :, 0:2, :])
                nc.scalar.dma_start(xb[:, 0, 0:1024], xT[:, 0, 0:1024])
                # bulk from the gpsimd queue, ordered by first-use time
                dma = nc.gpsimd.dma_start
                dma(xb[:, 0, 1024:2048], xT[:, 0, 1024:2048])
                dma(wqk_b[1][:, 0:2, :], wqk[1, :, 0:2, :])
                dma(wqk_b[0][:, 2:KO, :], wqk[0, :, 2:KO, :])
                dma(wqk_b[1][:, 2:KO, :], wqk[1, :, 2:KO, :])
                dma(xb[:, 1, :], xT[:, 1, :])
                dma(cos_b[:], cos_d.ap())
                dma(sin_b[:], sin_d.ap())
                for ko in range(2, KO):
                    dma(xb[:, ko, :], xT[:, ko, :])
                    # stage the remaining weight groups between x chunks,
                    # each well before its first use at ~13.6*(m-1) us
                    if ko in (5, 8, 11, 13):
                        m = {5: 2, 8: 3, 11: 4, 13: 5}[ko]
                        dma(wqk_b[m][:], wqk[m])
                dma(wqk_b[6][:], wqk[6])
                dma(wqk_b[7][:], wqk[7])
                dma(wv_b[:], wv_d.ap())
                dma(tmpl_sb[:], tmpl_d.ap())
                dma(wp_b[:], wp_d.ap())

                def qk_evac_rope(m, pss):
                    for n in range(NQ):
                        nc.scalar.activation(
                            raw[m][:, n * TQ:(n + 1) * TQ], pss[n][:], AF.Copy)
                    r = raw[m]
                    sh = shufp.tile([P, T], BF16, tag="shuf", name="sh")
                    nc.sync.dma_start(sh[0:64, :], r[64:128, :])
                    nc.sync.dma_start(sh[64:128, :], r[0:64, :])
                    nc.vector.tensor_mul(sh[:], sh[:], sin_b[:])
                    nc.vector.tensor_mul(r[:], r[:], cos_b[:])
                    nc.vector.tensor_add(r[:], r[:], sh[:])

                # m=0,1 interleaved: each x chunk feeds 8 matmuls, matching
                # the DMA supply rate during the cold start
                pss01 = [[psB.tile([P, TQ], F32, tag="psBig", name="psqk")
                          for _ in range(NQ)] for _ in range(2)]
                for ko in range(KO):
                    for g in range(2):
                        for n in range(NQ):
                            nc.tensor.matmul(
                                pss01[g][n][:], lhsT=wqk_b[g][:, ko, :],
                                rhs=xb[:, ko, n * TQ:(n + 1) * TQ],
                                start=(ko == 0), stop=(ko == KO - 1))
                qk_evac_rope(0, pss01[0])
                qk_evac_rope(1, pss01[1])

                for m in range(2, 8):
                    pss = [psB.tile([P, TQ], F32, tag="psBig", name="psqk")
                           for _ in range(NQ)]
                    for ko in range(KO):
                        for n in range(NQ):
                            nc.tensor.matmul(
                                pss[n][:], lhsT=wqk_b[m][:, ko, :],
                                rhs=xb[:, ko, n * TQ:(n + 1) * TQ],
                                start=(ko == 0), stop=(ko == KO - 1))
                    qk_evac_rope(m, pss)

                # v matmuls (natural layout), single merged evac per t
                for t in range(NT):
                    psv = psB.tile([P, TQ], F32, tag="psBig", name="psv")
                    for ko in range(KO):
                        nc.tensor.matmul(
                            psv[:], lhsT=xb[:, ko, t * P:(t + 1) * P],
                            rhs=wv_b[:, ko, :],
                            start=(ko == 0), stop=(ko == KO - 1))
                    nc.scalar.activation(v_b[:, t, :], psv[:], AF.Copy)

            # ========= Phase D+F: attention with woven projection =========
            with tc.tile_pool(name="attn", bufs=1) as ap_, \
                 tc.tile_pool(name="etp", bufs=3) as etp, \
                 tc.tile_pool(name="nrm", bufs=2) as nrm, \
                 tc.tile_pool(name="ystg", bufs=2) as ystg:

                outT = [ap_.tile([P, T], BF16, tag=f"outT{h}", name=f"outT{h}")
                        for h in range(NH)]

                with tc.tile_pool(name="psS", bufs=2, space="PSUM") as psS, \
                     tc.tile_pool(name="psO", bufs=2, space="PSUM") as psO, \
                     tc.tile_pool(name="psR", bufs=2, space="PSUM") as psR:

                    def proj_half(t, half, ys):
                        # y tile [t*P:(t+1)*P, half*1024:(half+1)*1024]
                        psp = psS.tile([P, 2, TQ], F32, tag="psscore",
                                       name="psy")
                        for h in range(NH):
                            for s in range(2):
                                cn = 2 * half + s
                                nc.tensor.matmul(
                                    psp[:, s, :],
                                    lhsT=outT[h][:, t * P:(t + 1) * P],
                                    rhs=wp_b[:, h, cn * TQ:(cn + 1) * TQ],
                                    start=(h == 0), stop=(h == NH - 1))
                        if half == 0:
                            nc.scalar.activation(ys[:, half], psp[:], AF.Copy)
                        else:
                            nc.vector.tensor_copy(ys[:, half], psp[:])
                        nc.sync.dma_start(
                            y[t * P:(t + 1) * P,
                              half * 2 * TQ:(half + 1) * 2 * TQ],
                            ys[:, half])

                    def attn_head(h, qo, weave_t):
                        # weave_t: projection t-tile of the previous qo-group
                        # emitted inside this head's S-matmul section
                        qr = raw[h]
                        kr = raw[4 + h]
                        vh = v_b[:, :, h * P:(h + 1) * P]
                        qsl = slice(qo * TQ, (qo + 1) * TQ)
                        nfull = 4 * qo
                        ys = None
                        if weave_t is not None:
                            ys = ystg.tile([P, 2, 2, TQ], F32, tag="ystage",
                                           name="ys")
                        ps_o = psO.tile([P, TQ], F32, tag="psout", name="ps_o")
                        # diag: S-matmuls for jr pairs; exp covers each pair
                        # in one activation (garbage columns zeroed by tmpl)
                        psd = psS.tile([P, 2, TQ], F32, tag="psscore",
                                       name="ps2d")
                        for jr in (0, 1):
                            j = nfull + jr
                            off = jr * P
                            nc.tensor.matmul(
                                psd[:, jr, off:TQ],
                                lhsT=kr[:, j * P:(j + 1) * P],
                                rhs=qr[:, qo * TQ + off:(qo + 1) * TQ],
                                start=True, stop=True)
                        et_d0 = etp.tile([P, 2, TQ], BF16, tag="et2",
                                         name="etd0", bufs=5)
                        nc.scalar.activation(et_d0[:], psd[:], AF.Exp,
                                             scale=SCALE)
                        nc.vector.tensor_mul(et_d0[:], et_d0[:],
                                             tmpl_sb[:, 0:2, :])
                        if weave_t is not None:
                            proj_half(weave_t, 0, ys)
                        psd2 = psS.tile([P, 2, TQ], F32, tag="psscore",
                                        name="ps2d2")
                        for jr in (2, 3):
                            j = nfull + jr
                            off = jr * P
                            nc.tensor.matmul(
                                psd2[:, jr - 2, off:TQ],
                                lhsT=kr[:, j * P:(j + 1) * P],
                                rhs=qr[:, qo * TQ + off:(qo + 1) * TQ],
                                start=True, stop=True)
                        et_d1 = etp.tile([P, 2, TQ], BF16, tag="et2",
                                         name="etd1", bufs=5)
                        nc.scalar.activation(et_d1[:, :, 2 * P:TQ],
                                             psd2[:, :, 2 * P:TQ], AF.Exp,
                                             scale=SCALE)
                        # only the written range [2P:TQ] of et_d1 is ever
                        # read; cols [0:2P) stay uninitialized
                        nc.vector.tensor_mul(et_d1[:, :, 2 * P:TQ],
                                             et_d1[:, :, 2 * P:TQ],
                                             tmpl_sb[:, 2:4, 2 * P:TQ])
                        # E accumulator for the softmax denominators
                        acc = etp.tile([P, TQ], BF16, tag="esum", name="esum",
                                       bufs=3)
                        nc.vector.tensor_add(acc[:], et_d0[:, 0, :],
                                             et_d0[:, 1, :])
                        nc.vector.tensor_add(acc[:, 2 * P:TQ],
                                             acc[:, 2 * P:TQ],
                                             et_d1[:, 0, 2 * P:TQ])
                        nc.vector.tensor_add(acc[:, 2 * P:TQ],
                                             acc[:, 2 * P:TQ],
                                             et_d1[:, 1, 2 * P:TQ])
                        if weave_t is not None:
                            proj_half(weave_t, 1, ys)
                        # diag O-matmuls
                        for jr in range(4):
                            j = nfull + jr
                            off = jr * P
                            etx = (et_d0, et_d1)[jr // 2]
                            nc.tensor.matmul(
                                ps_o[:, off:TQ], lhsT=vh[:, j, :],
                                rhs=etx[:, jr % 2, off:TQ],
                                start=(jr == 0),
                                stop=(qo == 0 and jr == 3))
                        # dense pairs with one-pair lookahead
                        npair = nfull // 2
                        pend = None
                        for pr in range(npair + 1):
                            if pr < npair:
                                ps2 = psS.tile([P, 2, TQ], F32,
                                               tag="psscore", name="ps2")
                                for s in range(2):
                                    j = 2 * pr + s
                                    nc.tensor.matmul(
                                        ps2[:, s, :],
                                        lhsT=kr[:, j * P:(j + 1) * P],
                                        rhs=qr[:, qsl],
                                        start=True, stop=True)
                                et2 = etp.tile([P, 2, TQ], BF16, tag="et2",
                                               name="et2", bufs=5)
                                nc.scalar.activation(et2[:], ps2[:],
                                                     AF.Exp, scale=SCALE)
                            else:
                                et2 = None
                            if pend is not None:
                                et2p, prp = pend
                                last = (prp == npair - 1)
                                for s in range(2):
                                    j = 2 * prp + s
                                    nc.tensor.matmul(
                                        ps_o[:], lhsT=vh[:, j, :],
                                        rhs=et2p[:, s, :],
                                        start=False, stop=(last and s == 1))
                                nc.vector.tensor_add(acc[:], acc[:],
                                                     et2p[:, 0, :])
                                nc.vector.tensor_add(acc[:], acc[:],
                                                     et2p[:, 1, :])
                            pend = (et2, pr) if et2 is not None else None
                        # single rowsum matmul; sums replicated on all
                        # partitions, reciprocal straight from PSUM
                        ps_r = psR.tile([P, TQ], F32, tag="psrow", name="ps_r")
                        nc.tensor.matmul(ps_r[:], lhsT=ones_sb[:], rhs=acc[:],
                                         start=True, stop=True)
                        recip = nrm.tile([P, TQ], F32, tag="recip",
                                         name="recip")
                        nc.vector.reciprocal_approx_fast(recip[:], ps_r[:])
                        nc.vector.tensor_mul(outT[h][:, qsl], ps_o[:],
                                             recip[:])

                    ready = []   # proj t-tiles produced by the previous group
                    for qo in reversed(range(NQ)):
                        for h in range(NH):
                            weave_t = ready.pop(0) if ready else None
                            attn_head(h, qo, weave_t)
                        ready = [4 * qo + i for i in range(4)]
                    # drain the final group's projection tiles (t=0..3)
                    for t in ready:
                        ys = ystg.tile([P, 2, 2, TQ], F32, tag="ystage",
                                       name="ys")
                        proj_half(t, 0, ys)
                        proj_half(t, 1, ys)

    nc.compile()
    return nc


def _get_nc():
    global _CACHED_NC
    if _CACHED_NC is None:
        _CACHED_NC = build_nc()
    return _CACHED_NC


LAST_RESULTS = None


def kernel(x, cos, sin, W_attn, W_proj):
    global LAST_RESULTS
    x = np.asarray(x, np.float32)
    cos = np.asarray(cos, np.float32)
    sin = np.asarray(sin, np.float32)
    W_attn = np.asarray(W_attn, np.float32)
    W_proj = np.asarray(W_proj, np.float32)
    B = x.shape[0]

    cosT = np.ascontiguousarray(cos.T).astype(BF)          # [D, T]
    sinTf = np.ascontiguousarray(sin.T).copy()
    sinTf[: D // 2] *= -1.0                                # sign-folded rotate
    sinT = sinTf.astype(BF)

    xTs = [np.ascontiguousarray(x[b].T).astype(BF) for b in range(B)]
    in_maps = []
    for b in range(B):
        for g in range(4):
            csl = slice(g * 512, (g + 1) * 512)
            wqk2 = np.concatenate([W_attn[:, csl], W_attn[:, C:][:, csl]],
                                  axis=1).astype(BF)       # [C, 1024]
            # pack [8, 128, 16, 128]: wqkr[m, p, ko, j] = wqk2[128*ko+p, 128*m+j]
            wqkr = np.ascontiguousarray(
                wqk2.reshape(KO, P, 8, P).transpose(2, 1, 0, 3))
            wv2 = W_attn[:, 2 * C:][:, csl].astype(BF)     # [C, 512]
            wvr = np.ascontiguousarray(
                wv2.reshape(KO, P, NH * D).transpose(1, 0, 2))  # [128,16,512]
            wp2 = W_proj[g * 512:(g + 1) * 512, :].astype(BF)   # [512, C]
            wpr = np.ascontiguousarray(
                wp2.reshape(NH, P, C).transpose(1, 0, 2))       # [128,4,2048]
            in_maps.append({"xT": xTs[b], "wqk": wqkr, "wv": wvr, "wp": wpr,
                            "cosT": cosT, "sinT": sinT})

    nc = _get_nc()
    res = run_bass_kernel_spmd(nc, in_maps, core_ids=list(range(8)),
                               trace=TRACE)
    LAST_RESULTS = res

    out = np.zeros((B, T, C), np.float32)
    for b in range(B):
        acc = res.results[b * 4 + 0]["y"].astype(np.float32)
        for g in range(1, 4):
            acc = acc + res.results[b * 4 + g]["y"]
        out[b] = acc
    return out
